# revision 1
# baseline (speedup 1.0000x reference)
"""Trainium2 Bass kernel for nn_Colorizer (retrieval_knn).

Pipeline (per sample, data-parallel over N=8 samples -> 8 cores):
  1. Patch-embed conv as matmul: featsT[c, p] = W[k, c]^T @ patchesT[k, p]
     (k = 8*8*3 = 192 patch pixels, p = 4 images * 32*32 patches = 4096)
  2. Similarity S[r, t] = refT[c, r]^T @ tgtT[c, t]   (r = 3072, t = 1024)
  3. E = exp(S - 50)  (softmax over r is shift-invariant; max|S| ~= 87 so
     the constant shift prevents fp32 exp overflow; underflow to 0 is safe)
  4. predT_unnorm = labels_aug^T @ E with labels_aug = [ones(16),
     zeros(16), labels(16)]: rows 0..15 = replicated softmax
     denominator, rows 32..47 = unnormalized predictions (zeros keep
     the blocks 32-partition-aligned; custom-DVE reciprocal requires
     partition base 0, standard ops handle base 32)
  5. Normalize: out = pred_rows * reciprocal(denom_rows), DMA out as
     [16, 1024]; host transposes to [1024, 16].

Host side only reshapes/transposes data (im2col layout + sharding); all
FLOPs run on device. Matmuls use float32r (TF32-like, full PE rate).

Perf notes (measured on HW):
  - PE clock needs ~5 us of sustained matmul work to leave the HAM
    throttle (1.2 -> 2.4 GHz); bf16 warm-up matmuls run during the DMA
    prologue so the real work starts warm.
  - pred matmuls are emitted two chunks behind their exp so the PE
    never waits on ACT (in-order PE queue would otherwise bubble).
  - conv PSUM->SBUF casts alternate DVE/ACT so the cast never gates PE.

Built on bacc.Bacc so compile() legalizes multi-semaphore waits (TRN2
instructions accept only one sync wait).
"""

import numpy as np

import concourse.mybir as mybir
from concourse import bacc
from concourse.bass_utils import run_bass_kernel_spmd
from concourse.tile import TileContext

F32 = mybir.dt.float32
F32R = mybir.dt.float32r
BF16 = mybir.dt.bfloat16

N = 8            # samples == cores
R_T, T_T = 3, 1  # ref / target frames
H = W_IMG = 256
C = 3
PATCH = 8
FEAT = 256
K_LAB = 16
HP = H // PATCH          # 32
PPI = HP * HP            # 1024 patches per image
NIMG = R_T + T_T         # 4
NPAT = NIMG * PPI        # 4096
KPIX = PATCH * PATCH * C  # 192
KPAD = 256               # K padded to 2x128 (K=64 matmuls run ~3x slow)
R = R_T * PPI            # 3072
T = T_T * PPI            # 1024
RC = R // 128            # 24 r-chunks
LABC = 48                # 16 ones cols, 16 zero cols, 16 label cols
LABW = RC * LABC         # swizzled label columns
EXP_SHIFT = -50.0
N_WARMUP = 18


def _build_nc():
    nc = bacc.Bacc(trn_type="TRN2", target_bir_lowering=False)

    pt_d = nc.declare_dram_parameter("pt", [KPIX, NPAT], F32R, isOutput=False)
    w_d = nc.declare_dram_parameter("w", [KPAD, FEAT], F32R, isOutput=False)
    lab_d = nc.declare_dram_parameter("lab", [128, RC * K_LAB], F32R, isOutput=False)
    out_d = nc.declare_dram_parameter("out", [K_LAB, T], F32, isOutput=True)

    with TileContext(nc) as tc:
        with (
            tc.tile_pool(name="const", bufs=1) as const,
            tc.tile_pool(name="feats", bufs=1) as feats,
            tc.tile_pool(name="mmps", bufs=2, space="PSUM") as mmps,
            tc.tile_pool(name="predps", bufs=1, space="PSUM") as predps,
            tc.tile_pool(name="wps", bufs=2, space="PSUM") as wpsp,
            tc.tile_pool(name="epool", bufs=4) as epool,
            tc.tile_pool(name="opool", bufs=2) as opool,
        ):
            # PE warm-up source: first DVE op so matmuls can start early
            wu_sb = const.tile([128, 512], BF16, tag="wu")
            nc.vector.memset(wu_sb, 0.0)

            # ---- input loads: 2 HWDGE rings (sync + scalar), tgt first ----
            w_sb0 = const.tile([128, FEAT], F32R, tag="w0")
            w_sb1 = const.tile([KPAD - 128, FEAT], F32R, tag="w1")
            nc.sync.dma_start(out=w_sb0, in_=w_d.ap()[0:128, :])
            nc.gpsimd.dma_start(out=w_sb1, in_=w_d.ap()[128:KPAD, :])

            shift_sb = const.tile([128, 1], F32, tag="shift")
            nc.vector.memset(shift_sb, EXP_SHIFT)

            pt_sb0 = const.tile([128, NPAT], F32R, tag="pt0")
            pt_sb1 = const.tile([KPAD - 128, NPAT], F32R, tag="pt1")
            # pad rows 64..127 of the K=128..255 tile with zeros on-chip
            nc.vector.memset(pt_sb1[64:128, :].bitcast(F32), 0.0)
            NB_ORDER = (3, 0, 1, 2)  # tgt image block first
            NBD = 4
            for nb in NB_ORDER:
                sl = slice(nb * (NPAT // NBD), (nb + 1) * (NPAT // NBD))
                nc.sync.dma_start(out=pt_sb0[:, sl], in_=pt_d.ap()[0:128, sl])
                nc.gpsimd.dma_start(
                    out=pt_sb1[0:KPIX - 128, sl], in_=pt_d.ap()[128:KPIX, sl]
                )

            lab_sb = const.tile([128, RC, 48], F32R, tag="lab")
            nc.gpsimd.memset(lab_sb[:, :, 0:16].bitcast(F32), 1.0)
            nc.gpsimd.memset(lab_sb[:, :, 16:32].bitcast(F32), 0.0)
            nc.gpsimd.dma_start(
                out=lab_sb[:, :, 32:48],
                in_=lab_d.ap().rearrange("p (rc k) -> p rc k", k=K_LAB),
            )

            # ---- PE clock warm-up during the DMA prologue (HAM) ----
            for _ in range(N_WARMUP):
                wps = wpsp.tile([128, 512], F32, tag="wp", name="wps")
                nc.tensor.matmul(wps, wu_sb[:, 0:128], wu_sb, start=True, stop=True)

            # ---- 1. conv: featsT[c, p] (c split in two 128-row tiles) ----
            f_sb = [
                feats.tile([128, NPAT], F32R, tag="f0", name="f_sb0"),
                feats.tile([128, NPAT], F32R, tag="f1", name="f_sb1"),
            ]
            NB = 4  # column blocks of 1024
            BW = NPAT // NB
            pred_ps = predps.tile([LABC, T], F32, tag="pred")

            def conv_block(nb):
                for cc in range(2):
                    ps = mmps.tile([128, BW], F32, tag="mm", name="ps")
                    csl = slice(cc * 128, (cc + 1) * 128)
                    for h in range(2):
                        hsl = slice(nb * BW + h * 512, nb * BW + (h + 1) * 512)
                        psl = slice(h * 512, (h + 1) * 512)
                        nc.tensor.matmul(
                            ps[:, psl], w_sb0[:, csl], pt_sb0[:, hsl],
                            start=True, stop=False,
                        )
                        nc.tensor.matmul(
                            ps[:, psl], w_sb1[:, csl], pt_sb1[:, hsl],
                            start=False, stop=True,
                        )
                    dst = f_sb[cc][:, nb * BW:(nb + 1) * BW]
                    # split the cast across DVE and ACT so neither gates PE
                    nc.vector.tensor_copy(dst[:, 0:512], ps[:, 0:512])
                    nc.scalar.copy(dst[:, 512:BW], ps[:, 512:BW])

            e_tiles = {}

            def s_part(rc):
                rsl = slice(rc * 128, (rc + 1) * 128)
                s_ps = mmps.tile([128, T], F32, tag="mm", name="s_ps")
                for th in range(2):
                    psl = slice(th * 512, (th + 1) * 512)
                    tsl = slice(R + th * 512, R + (th + 1) * 512)
                    nc.tensor.matmul(
                        s_ps[:, psl], f_sb[0][:, rsl], f_sb[0][:, tsl],
                        start=True, stop=False,
                    )
                    nc.tensor.matmul(
                        s_ps[:, psl], f_sb[1][:, rsl], f_sb[1][:, tsl],
                        start=False, stop=True,
                    )
                e_sb = epool.tile([128, T], F32R, tag="e", name="e_sb")
                nc.scalar.activation(
                    e_sb, s_ps, mybir.ActivationFunctionType.Exp,
                    bias=shift_sb, scale=1.0,
                )
                e_tiles[rc] = e_sb

            def pred_part(rc):
                e_sb = e_tiles.pop(rc)
                for th in range(2):
                    psl = slice(th * 512, (th + 1) * 512)
                    nc.tensor.matmul(
                        pred_ps[:, psl],
                        lab_sb[:, rc, :],
                        e_sb[:, psl],
                        start=(rc == 0), stop=(rc == RC - 1),
                    )

            # conv blocks feed S chunks; pred lags two chunks behind its exp
            PRED_LAG = 2
            emitted = []

            def emit_s(rc):
                s_part(rc)
                emitted.append(rc)
                if len(emitted) > PRED_LAG:
                    pred_part(emitted[len(emitted) - 1 - PRED_LAG])

            conv_block(3)
            conv_block(0)
            for rc in range(0, 8):
                emit_s(rc)
            conv_block(1)
            for rc in range(8, 16):
                emit_s(rc)
            conv_block(2)
            for rc in range(16, 24):
                emit_s(rc)
            for rc in emitted[-PRED_LAG:]:
                pred_part(rc)

            # ---- 5. normalize label rows by replicated denom rows ----
            rec = opool.tile([K_LAB, T], F32, tag="rec")
            nc.vector.reciprocal_approx_fast(rec, pred_ps[0:K_LAB, :])
            o_sb = opool.tile([K_LAB, T], F32, tag="o")
            nc.vector.tensor_mul(o_sb, pred_ps[32:32 + K_LAB, :], rec)
            nc.sync.dma_start(out=out_d.ap(), in_=o_sb)

    nc.compile()
    return nc


_NC_CACHE = None


def _get_nc():
    global _NC_CACHE
    if _NC_CACHE is None:
        _NC_CACHE = _build_nc()
    return _NC_CACHE


def prep_in_maps(reference_images, target_images, reference_labels, w_feat):
    """Host-side sharding + layout prep (no arithmetic)."""
    ri = np.ascontiguousarray(reference_images, dtype=np.float32)
    ti = np.ascontiguousarray(target_images, dtype=np.float32)
    lab = np.ascontiguousarray(reference_labels, dtype=np.float32)
    wf = np.ascontiguousarray(w_feat, dtype=np.float32)

    w2 = np.zeros((KPAD, FEAT), np.float32)
    w2[:KPIX] = wf.reshape(KPIX, FEAT)
    imgs = np.concatenate([ri, ti], axis=1)  # [N, 4, H, W, C]
    # patchesT[n] : [(dy dx ch), (img py px)]
    ptT = np.ascontiguousarray(
        imgs.reshape(N, NIMG, HP, PATCH, HP, PATCH, C)
        .transpose(0, 3, 5, 6, 1, 2, 4)
        .reshape(N, KPIX, NPAT)
    )
    lab_sw = np.ascontiguousarray(
        lab.reshape(N, RC, 128, K_LAB).transpose(0, 2, 1, 3).reshape(N, 128, RC * K_LAB)
    )
    return [
        {"pt": ptT[n], "w": w2, "lab": lab_sw[n]} for n in range(N)
    ]


def run(in_maps, **kwargs):
    nc = _get_nc()
    return run_bass_kernel_spmd(nc, in_maps, list(range(N)), **kwargs)


def kernel(reference_images, target_images, reference_labels, w_feat):
    in_maps = prep_in_maps(
        reference_images, target_images, reference_labels, w_feat
    )
    res = run(in_maps)
    # device emits [16, T]; transpose to [T, 16] here (pure layout)
    out = np.stack(
        [np.ascontiguousarray(res.results[n]["out"].T) for n in range(N)]
    )
    return out.reshape(N, T_T, HP, HP, K_LAB)



# revision 2
# speedup vs baseline: 1.0444x; 1.0444x over previous
"""Trainium2 Bass kernel for nn_Colorizer (retrieval_knn) — v2.

Pipeline (per sample, data-parallel over N=8 samples -> 8 cores):
  S[r,t] = ref_p^T (W W^T) tgt_p  (Gram trick: the ref-side conv is
  eliminated; only G = W W^T [192x192] and Gt = G @ tgt_p [192,1024]
  are computed before the big S matmul).
  E = exp(S - 50); pred/denom via one col-tiled matmul with
  lab_aug = [ones(16) | labels(16)] per 32-col group, each group
  covering one t-quarter; out = pred * reciprocal(denom).

Device dtypes: patches/G/Gt in fp16 (same 10-bit mantissa as the
fp32r baseline; halves input DMA), E/labels in fp32r, PSUM fp32.

Layout notes:
  pt [192, 4096] fp16, cols = [tgt(1024) | ref(3072)]; rows 192:256
  zero-padded on chip so every contraction is K=128.
  wt [128, 384] f32: W^T packed as [c-chunk0 | c-chunk1] columns.
  Col-tiled pred: group j (tile_position (0,32j)) accumulates
  denom rows 32j:32j+16 and pred rows 32j+16:32j+32 for t-quarter j.
"""

import numpy as np

import concourse.mybir as mybir
from concourse import bacc
from concourse.bass_utils import run_bass_kernel_spmd
from concourse.tile import TileContext

F32 = mybir.dt.float32
F32R = mybir.dt.float32r
F16 = mybir.dt.float16
BF16 = mybir.dt.bfloat16

N = 8            # samples == cores
R_T, T_T = 3, 1  # ref / target frames
H = W_IMG = 256
C = 3
PATCH = 8
FEAT = 256
K_LAB = 16
HP = H // PATCH          # 32
PPI = HP * HP            # 1024 patches per image
NIMG = R_T + T_T         # 4
NPAT = NIMG * PPI        # 4096
KPIX = PATCH * PATCH * C  # 192
R = R_T * PPI            # 3072
T = T_T * PPI            # 1024
RC = R // 128            # 24 r-chunks
TQ = T // 4              # 256 cols per pred col-group
EXP_SHIFT = -50.0
N_WARMUP = 18
PRED_LAG = 2


def _build_nc():
    nc = bacc.Bacc(trn_type="TRN2", target_bir_lowering=False)

    pt_d = nc.declare_dram_parameter("pt", [KPIX, NPAT], F16, isOutput=False)
    wt_d = nc.declare_dram_parameter("wt", [128, 384], F32R, isOutput=False)
    lab_d = nc.declare_dram_parameter("lab", [128, RC * K_LAB], F32R, isOutput=False)
    out_d = nc.declare_dram_parameter("out", [K_LAB, T], F32, isOutput=True)

    with TileContext(nc) as tc:
        with (
            tc.tile_pool(name="const", bufs=1) as const,
            tc.tile_pool(name="epool", bufs=4) as epool,
            tc.tile_pool(name="predps", bufs=1, space="PSUM") as predps,
        ):
            # ---- SBUF tiles ----
            wu_sb = const.tile([128, 256], BF16, tag="wu")
            nc.vector.memset(wu_sb.bitcast(F32), 0.0)
            shift_sb = const.tile([128, 1], F32, tag="shift")
            nc.vector.memset(shift_sb, EXP_SHIFT)

            wt_sb = const.tile([128, 384], F32R, tag="wt")
            pt_sb0 = const.tile([128, NPAT], F16, tag="pt0")
            pt_sb1 = const.tile([128, NPAT], F16, tag="pt1")
            g_sb0 = const.tile([128, 192], F16, tag="g0")
            g_sb1 = const.tile([128, 192], F16, tag="g1")
            gt_sb0 = const.tile([128, T], F16, tag="gt0")
            gt_sb1 = const.tile([128, T], F16, tag="gt1")
            # [ones16 | zeros16 | lab16] per rc: denom rows 0:16, pred
            # rows 32:48 (zeros keep pred 32-partition-aligned)
            comb_sb = const.tile([128, RC, 48], F32R, tag="comb")
            rec_sb = const.tile([K_LAB, T], F32, tag="rec")
            o_sb = const.tile([K_LAB, T], F32, tag="o")
            dummy_sb = const.tile([128, 1], F32, tag="dummy")

            # ---- input DMA: two HWDGE rings, ordered by need time ----
            # sync: W^T (G), tgt row-chunk0 (Gt), then ref row-chunk0 in
            # two halves so early S chunks unblock before the full load
            nc.sync.dma_start(out=wt_sb, in_=wt_d.ap())
            nc.sync.dma_start(out=pt_sb0[:, 0:PPI], in_=pt_d.ap()[0:128, 0:PPI])
            nc.sync.dma_start(
                out=pt_sb0[:, PPI:PPI + R // 2],
                in_=pt_d.ap()[0:128, PPI:PPI + R // 2],
            )
            nc.sync.dma_start(
                out=pt_sb0[:, PPI + R // 2:NPAT],
                in_=pt_d.ap()[0:128, PPI + R // 2:NPAT],
            )
            # scalar: tgt row-chunk1, ref row-chunk1 (2 halves), labels
            nc.scalar.dma_start(
                out=pt_sb1[0:64, 0:PPI], in_=pt_d.ap()[128:KPIX, 0:PPI]
            )
            nc.scalar.dma_start(
                out=pt_sb1[0:64, PPI:PPI + R // 2],
                in_=pt_d.ap()[128:KPIX, PPI:PPI + R // 2],
            )
            # pull the exp ACT table load off the critical path
            nc.scalar.activation(
                dummy_sb, shift_sb, mybir.ActivationFunctionType.Exp,
                bias=0.0, scale=1.0,
            )
            nc.scalar.dma_start(
                out=pt_sb1[0:64, PPI + R // 2:NPAT],
                in_=pt_d.ap()[128:KPIX, PPI + R // 2:NPAT],
            )
            nc.scalar.dma_start(
                out=comb_sb[:, :, 32:48],
                in_=lab_d.ap().rearrange("p (rc k) -> p rc k", k=K_LAB),
            )

            # ---- on-chip zero-padding + constants ----
            nc.vector.memset(pt_sb1[64:128, :].bitcast(F32), 0.0)
            nc.gpsimd.memset(g_sb1[64:128, :].bitcast(F32), 0.0)
            nc.gpsimd.memset(gt_sb1[64:128, :].bitcast(F32), 0.0)
            nc.gpsimd.memset(comb_sb[:, :, 0:16].bitcast(F32), 1.0)
            nc.gpsimd.memset(comb_sb[:, :, 16:32].bitcast(F32), 0.0)

            # ---- PE warm-up during the DMA prologue (HAM) ----
            with tc.tile_pool(name="wps", bufs=2, space="PSUM") as wpsp:
                for _ in range(N_WARMUP):
                    wps = wpsp.tile([128, 256], F32, tag="wp", name="wps")
                    nc.tensor.matmul(
                        wps, wu_sb[:, 0:128], wu_sb, start=True, stop=True
                    )

                # ---- G = W W^T  (fp32r), cast to fp16 ----
                # G_ps0 [k1 0:128, k2 0:192], G_ps1 [k1 128:192, k2 0:192]
                g_ps0 = wpsp.tile([128, 192], F32, tag="gps0")
                g_ps1 = wpsp.tile([64, 192], F32, tag="gps1")
                for cc in range(2):
                    csl = slice(cc * 192, cc * 192 + 192)
                    nc.tensor.matmul(
                        g_ps0, wt_sb[:, cc * 192:cc * 192 + 128],
                        wt_sb[:, csl], start=(cc == 0), stop=(cc == 1),
                    )
                    nc.tensor.matmul(
                        g_ps1, wt_sb[:, cc * 192 + 128:cc * 192 + 192],
                        wt_sb[:, csl], start=(cc == 0), stop=(cc == 1),
                    )
                nc.vector.tensor_copy(g_sb0[:, 0:192], g_ps0)
                nc.vector.tensor_copy(g_sb1[0:64, 0:192], g_ps1)

            # ---- Gt = G @ tgt_p  (fp16), cast to fp16 ----
            with tc.tile_pool(name="gtps", bufs=1, space="PSUM") as gtpsp:
                gt_ps0 = gtpsp.tile([128, T], F32, tag="gtps0")
                gt_ps1 = gtpsp.tile([64, T], F32, tag="gtps1")
                g_chunks = (g_sb0, g_sb1)
                for m, (gt_ps, msl) in enumerate(
                    ((gt_ps0, slice(0, 128)), (gt_ps1, slice(128, 192)))
                ):
                    for k2c in range(2):
                        for th in range(2):
                            tsl = slice(th * 512, (th + 1) * 512)
                            nc.tensor.matmul(
                                gt_ps[:, tsl],
                                g_chunks[k2c][:, msl],
                                (pt_sb0 if k2c == 0 else pt_sb1)[:, tsl],
                                start=(k2c == 0), stop=(k2c == 1),
                            )
                nc.vector.tensor_copy(gt_sb0, gt_ps0)
                nc.scalar.copy(gt_sb1[0:64, :], gt_ps1)

            # ---- main loop: S chunks -> exp -> pred ----
            pred_ps = predps.tile([48, T], F32, tag="pred")
            e_tiles = {}

            def s_part(rc):
                rsl = slice(PPI + rc * 128, PPI + (rc + 1) * 128)
                s_ps = spsp.tile([128, T], F32, tag="s", name="s_ps")
                for kc, (pt_sb, gt_sb) in enumerate(
                    ((pt_sb0, gt_sb0), (pt_sb1, gt_sb1))
                ):
                    for th in range(2):
                        tsl = slice(th * 512, (th + 1) * 512)
                        nc.tensor.matmul(
                            s_ps[:, tsl], pt_sb[:, rsl], gt_sb[:, tsl],
                            start=(kc == 0), stop=(kc == 1),
                        )
                e_sb = epool.tile([128, T], F32R, tag="e", name="e_sb")
                nc.scalar.activation(
                    e_sb, s_ps, mybir.ActivationFunctionType.Exp,
                    bias=shift_sb, scale=1.0,
                )
                e_tiles[rc] = e_sb

            def pred_part(rc):
                e_sb = e_tiles.pop(rc)
                for th in range(2):
                    psl = slice(th * 512, (th + 1) * 512)
                    nc.tensor.matmul(
                        pred_ps[:, psl], comb_sb[:, rc, :], e_sb[:, psl],
                        start=(rc == 0), stop=(rc == RC - 1),
                    )

            with tc.tile_pool(name="sps", bufs=3, space="PSUM") as spsp:
                for rc in range(RC):
                    s_part(rc)
                    if rc >= PRED_LAG:
                        pred_part(rc - PRED_LAG)
                for rc in range(RC - PRED_LAG, RC):
                    pred_part(rc)

                # ---- normalize: out = pred rows * 1/denom rows ----
                nc.vector.reciprocal_approx_fast(rec_sb, pred_ps[0:K_LAB, :])
                nc.vector.tensor_mul(o_sb, pred_ps[32:32 + K_LAB, :], rec_sb)
                nc.sync.dma_start(out=out_d.ap(), in_=o_sb)

    nc.compile()
    return nc


_NC_CACHE = None


def _get_nc():
    global _NC_CACHE
    if _NC_CACHE is None:
        _NC_CACHE = _build_nc()
    return _NC_CACHE


def prep_in_maps(reference_images, target_images, reference_labels, w_feat):
    """Host-side sharding + layout prep (no arithmetic)."""
    ri = np.ascontiguousarray(reference_images, dtype=np.float32)
    ti = np.ascontiguousarray(target_images, dtype=np.float32)
    lab = np.ascontiguousarray(reference_labels, dtype=np.float32)
    wf = np.ascontiguousarray(w_feat, dtype=np.float32)

    # W^T [256 c, 192 k] packed as [128, 384]: cols 0:192 = c rows 0:128
    w2 = wf.reshape(KPIX, FEAT)            # [k, c]
    wt = np.ascontiguousarray(w2.T)        # [c, k]
    wt2 = np.concatenate([wt[0:128], wt[128:256]], axis=1)  # [128, 384]

    imgs = np.concatenate([ti, ri], axis=1)  # [N, 4, H, W, C], tgt FIRST
    ptT = np.ascontiguousarray(
        imgs.reshape(N, NIMG, HP, PATCH, HP, PATCH, C)
        .transpose(0, 3, 5, 6, 1, 2, 4)
        .reshape(N, KPIX, NPAT)
        .astype(np.float16)
    )
    lab_sw = np.ascontiguousarray(
        lab.reshape(N, RC, 128, K_LAB).transpose(0, 2, 1, 3).reshape(N, 128, RC * K_LAB)
    )
    return [
        {"pt": ptT[n], "wt": wt2, "lab": lab_sw[n]} for n in range(N)
    ]


def run(in_maps, **kwargs):
    nc = _get_nc()
    return run_bass_kernel_spmd(nc, in_maps, list(range(N)), **kwargs)


def kernel(reference_images, target_images, reference_labels, w_feat):
    in_maps = prep_in_maps(
        reference_images, target_images, reference_labels, w_feat
    )
    res = run(in_maps)
    # device emits [16, T]; transpose to [T, 16] here (pure layout)
    out = np.stack(
        [np.ascontiguousarray(res.results[n]["out"].T) for n in range(N)]
    )
    return out.reshape(N, T_T, HP, HP, K_LAB)
